# revision 4
# baseline (speedup 1.0000x reference)
"""CorrFPNHead Trainium kernel: 8-core data-parallel (batch x row-quarters).

Sharding: core = b*4 + q. Each core computes output rows [q*H/4, (q+1)*H/4)
of batch sample b for all 4 pyramid levels. Inputs are sliced with
zero-padded halos on the host (x: +-2 rows; ref: +-10 rows and +-8 cols,
which also realizes the correlation's zero padding), so every core runs an
identical program with no cross-core communication.

Correlation per vertical displacement di is a Gram matrix
G[di,h,w,z] = sum_c x[c,h,w] * ref[c,h+di-8,z]  (one batched matmul on PE),
and the 17 horizontal displacements are the diagonals z = w+dj, extracted
with strided slices over the flattened (w,z) axis at stride Z+1 -- pure
data movement, exact, no gather ops.
"""

import numpy as np
import jax
import jax.numpy as jnp
from jax import lax

PATCH = 17
B, C = 2, 256
SIZES = [(128, 128), (64, 64), (32, 32), (16, 16)]
NQ = 4
NCORES = 8


def _conv3x3_validh(x, w, b):
    # x: [Cin, Hin, W]; VALID in h (input rows carry the +-1 halo), SAME in w.
    y = lax.conv_general_dilated(
        x[None], w, window_strides=(1, 1), padding=((0, 0), (1, 1)),
        dimension_numbers=('NCHW', 'OIHW', 'NCHW'),
    )[0]
    return y + b[:, None, None]


def _corr(x, r):
    """x: [C, Rx, W]; r: [C, Rx+16, W+16]. Returns [289, Rx, W], d=di*17+dj."""
    Cc, Rx, W = x.shape
    Z = W + 16
    rstack = jnp.stack(
        [lax.slice_in_dim(r, di, di + Rx, axis=1) for di in range(PATCH)], axis=0
    )  # [17, C, Rx, Z]
    g = jnp.einsum('chw,dchz->dhwz', x, rstack,
                   preferred_element_type=jnp.float32)  # [17, Rx, W, Z]
    gf = g.reshape(PATCH, Rx, W * Z)
    lim = (W - 1) * (Z + 1)
    diags = [
        lax.slice(gf, (0, 0, dj), (PATCH, Rx, dj + lim + 1), (1, 1, Z + 1))
        for dj in range(PATCH)
    ]  # each [17di, Rx, W]
    corr = jnp.stack(diags, axis=1)  # [di, dj, Rx, W]
    return corr.reshape(PATCH * PATCH, Rx, W)


def _level_fn(x, r, wm, bm):
    corr = _corr(x, r)
    r_in = r[:, 8:-8, 8:-8]
    feat = jnp.concatenate([x, r_in, corr], axis=0)  # [801, Rx, W]
    return _conv3x3_validh(feat, wm, bm)             # [256, Rx-2, W]


def _upsample2(x):
    c, h, w = x.shape
    return jnp.broadcast_to(x[:, :, None, :, None], (c, h, 2, w, 2)).reshape(c, 2 * h, 2 * w)


def _top_fn(m0, m1, m2, m3, k0, k1, k2, k3,
            wo0, bo0, wo1, bo1, wo2, bo2, wo3, bo3):
    # k{l}: [Rm_l] row-validity masks -- zero the map halo rows that fall
    # outside the image (they emulate the reference conv's zero padding).
    maps = [m0 * k0[None, :, None], m1 * k1[None, :, None],
            m2 * k2[None, :, None], m3 * k3[None, :, None]]
    for l in (2, 1, 0):
        up = _upsample2(maps[l + 1])
        maps[l] = maps[l] + up[:, 1:1 + maps[l].shape[1], :]
    wos = [wo0, wo1, wo2, wo3]
    bos = [bo0, bo1, bo2, bo3]
    return tuple(_conv3x3_validh(maps[l], wos[l], bos[l]) for l in range(4))


_JITS = None


def _get_jits():
    global _JITS
    if _JITS is None:
        _JITS = ([jax.jit(_level_fn) for _ in range(4)], jax.jit(_top_fn))
    return _JITS


def _slice_pad_rows(a, lo, hi):
    Cc, H, W = a.shape
    out = np.zeros((Cc, hi - lo, W), a.dtype)
    s, e = max(lo, 0), min(hi, H)
    if e > s:
        out[:, s - lo:e - lo, :] = a[:, s:e, :]
    return out


def kernel(**inputs):
    xs = [np.asarray(inputs[f'x{l}']) for l in range(4)]
    rxs = [np.asarray(inputs[f'rx{l}']) for l in range(4)]
    devs = jax.devices()[:NCORES]
    level_jits, top_jit = _get_jits()

    wm = [np.asarray(inputs[f'wm{l}']) for l in range(4)]
    bm = [np.asarray(inputs[f'bm{l}']) for l in range(4)]
    wo = [np.asarray(inputs[f'wo{l}']) for l in range(4)]
    bo = [np.asarray(inputs[f'bo{l}']) for l in range(4)]

    # stage per-core inputs
    core_args = []
    for core in range(NCORES):
        b, q = divmod(core, NQ)
        d = devs[core]
        largs = []
        for l, (H, W) in enumerate(SIZES):
            R = H // NQ
            s, e = q * R, (q + 1) * R
            xsl = _slice_pad_rows(xs[l][b], s - 2, e + 2)
            rsl = _slice_pad_rows(rxs[l][b], s - 10, e + 10)
            rsl = np.pad(rsl, ((0, 0), (0, 0), (8, 8)))
            largs.append((
                jax.device_put(xsl, d), jax.device_put(rsl, d),
                jax.device_put(wm[l], d), jax.device_put(bm[l], d),
            ))
        kargs = []
        for l, (H, W) in enumerate(SIZES):
            R = H // NQ
            k = np.ones(R + 2, np.float32)
            if q == 0:
                k[0] = 0.0
            if q == NQ - 1:
                k[-1] = 0.0
            kargs.append(jax.device_put(k, d))
        wargs = []
        for l in range(4):
            wargs += [jax.device_put(wo[l], d), jax.device_put(bo[l], d)]
        core_args.append((largs, kargs, wargs))

    # dispatch: all maps first (async across devices), then top
    futs = []
    for core in range(NCORES):
        largs, kargs, wargs = core_args[core]
        ms = [level_jits[l](*largs[l]) for l in range(4)]
        futs.append(top_jit(*ms, *kargs, *wargs))

    outs = [np.zeros((B, C, H, W), np.float32) for (H, W) in SIZES]
    for core in range(NCORES):
        b, q = divmod(core, NQ)
        res = futs[core]
        for l, (H, W) in enumerate(SIZES):
            R = H // NQ
            s, e = q * R, (q + 1) * R
            outs[l][b, :, s:e, :] = np.asarray(res[l])
    return tuple(outs)
